# revision 9
# baseline (speedup 1.0000x reference)
"""Trainium2 Bass kernel for nn_MmbeddingsEncoder (segment_reduce).

Strategy (data-parallel over 8 NeuronCores):
  - rows (N=1e6) sharded 8-way; each core runs the 2-layer MLP on its shard
    (bf16 stationary-weight matmuls on PE),
  - local segment sums+counts via GPSIMD scatter_add into bf16 accumulators
    [80ch, 8192, 2] (ch 0..63 = z1 features, ch 64..79 = ones -> counts;
    2 d-slots split rows by parity to halve bf16 accumulation depth),
  - fp32 ReduceScatter over the 8 cores (each core ends up owning 1024
    segments), then the small dense head (divide, mean/logvar projections,
    reparameterized sample) on the owned q-shard,
  - host concatenates the 8 output shards.

Host-side work is limited to data-independent layout/dtype transforms
(sharding, padding, transpose, one-hot-free int16 repack).
"""

import numpy as np
import ml_dtypes

from contextlib import ExitStack

from concourse import bass, mybir, tile, bacc
from concourse.bass_utils import run_bass_kernel_spmd
from concourse.masks import make_identity

BF16 = mybir.dt.bfloat16
F32 = mybir.dt.float32
I16 = mybir.dt.int16

# problem constants (hardcoded per contract)
N = 1_000_000
D_IN = 64
H0, H1 = 128, 64
Q = 8192
D = 16
N_CORES = 8

R = N // N_CORES              # rows per core = 125000
CHUNK = 8192                  # rows per scatter_add call
N_CHUNK = 16
RP = CHUNK * N_CHUNK          # padded rows per core = 131072
QS = Q // N_CORES             # q-shard per core = 1024

SUB = 2048                    # xyt DMA subchunk (columns)
MM = 512                      # matmul free-dim slab


def build_program(n_cores=N_CORES, rp=RP, n_chunk=N_CHUNK, q=Q, qs=None):
    """Build the SPMD Bass program. Returns (nc, names dict)."""
    if qs is None:
        qs = q // n_cores
    chunk = rp // n_chunk
    nsub = chunk // SUB
    nmm = SUB // MM

    nc = bacc.Bacc("TRN2", target_bir_lowering=False, debug=False,
                   num_devices=n_cores)

    # ---- I/O ----
    xyt = nc.dram_tensor("xyt", [D_IN + 1, rp], BF16, kind="ExternalInput")
    idsw = [nc.dram_tensor(f"idsw{s}", [16, rp // 16], I16, kind="ExternalInput")
            for s in range(2)]
    w0 = nc.dram_tensor("w0", [D_IN + 1, H0], BF16, kind="ExternalInput")
    b0 = nc.dram_tensor("b0", [H0, 1], F32, kind="ExternalInput")
    w1 = nc.dram_tensor("w1", [H0, H1], BF16, kind="ExternalInput")
    b1 = nc.dram_tensor("b1", [H1, 1], F32, kind="ExternalInput")
    wm = [nc.dram_tensor(f"wm{s}", [H1, D], F32, kind="ExternalInput") for s in range(2)]
    bm = [nc.dram_tensor(f"bm{s}", [D, 1], F32, kind="ExternalInput") for s in range(2)]
    wv = [nc.dram_tensor(f"wv{s}", [H1, D], F32, kind="ExternalInput") for s in range(2)]
    bv = [nc.dram_tensor(f"bv{s}", [D, 1], F32, kind="ExternalInput") for s in range(2)]
    epst = [nc.dram_tensor(f"epst{s}", [D, qs], F32, kind="ExternalInput")
            for s in range(2)]
    out = nc.dram_tensor("out", [6, qs, D], F32, kind="ExternalOutput")

    AF = mybir.ActivationFunctionType
    OP = mybir.AluOpType

    with tile.TileContext(nc) as tc, ExitStack() as ctx:
        const = ctx.enter_context(tc.tile_pool(name="const", bufs=1))
        acc_pool = ctx.enter_context(tc.tile_pool(name="acc", bufs=1))
        ids_pool = ctx.enter_context(tc.tile_pool(name="ids", bufs=1))
        phase1 = ExitStack()
        xy_pool = phase1.enter_context(tc.tile_pool(name="xy", bufs=3))
        ht_pool = phase1.enter_context(tc.tile_pool(name="ht", bufs=3))
        add_pool = phase1.enter_context(tc.tile_pool(name="addt", bufs=1))
        ps1 = phase1.enter_context(tc.tile_pool(name="ps1", bufs=2, space="PSUM"))
        ps2 = phase1.enter_context(tc.tile_pool(name="ps2", bufs=2, space="PSUM"))

        # ---- constants / weights ----
        w0t = const.tile([D_IN + 1, H0], BF16)
        nc.sync.dma_start(out=w0t[:], in_=w0[:, :])
        b0t = const.tile([H0, 1], F32)
        nc.sync.dma_start(out=b0t[:], in_=b0[:, :])
        w1t = const.tile([H0, H1], BF16)
        nc.sync.dma_start(out=w1t[:], in_=w1[:, :])
        b1t = const.tile([H1, 1], F32)
        nc.sync.dma_start(out=b1t[:], in_=b1[:, :])
        wmt = [const.tile([H1, D], F32, name=f"wmt{s}") for s in range(2)]
        wvt = [const.tile([H1, D], F32, name=f"wvt{s}") for s in range(2)]
        bmt = [const.tile([D, 1], F32, name=f"bmt{s}") for s in range(2)]
        bvt = [const.tile([D, 1], F32, name=f"bvt{s}") for s in range(2)]
        for s in range(2):
            nc.sync.dma_start(out=wmt[s][:], in_=wm[s][:, :])
            nc.sync.dma_start(out=wvt[s][:], in_=wv[s][:, :])
            nc.sync.dma_start(out=bmt[s][:], in_=bm[s][:, :])
            nc.sync.dma_start(out=bvt[s][:], in_=bv[s][:, :])
        epstt = [const.tile([D, qs], F32, name=f"epstt{s}") for s in range(2)]
        for s in range(2):
            nc.sync.dma_start(out=epstt[s][:], in_=epst[s][:, :])
        ones64 = const.tile([1, H1], F32)
        nc.vector.memset(ones64[:], 1.0)
        ident = const.tile([128, 128], F32)
        make_identity(nc, ident[:])

        # ---- ids, replicated into each 16-partition group ----
        idst = [ids_pool.tile([128, rp // 16], I16, name=f"idst{s}") for s in range(2)]
        for s in range(2):
            for g in range(8):
                nc.sync.dma_start(out=idst[s][16 * g:16 * (g + 1), :],
                                  in_=idsw[s][:, :])

        # ---- accumulators (bf16), [80, q, 2] flattened ----
        acc = [acc_pool.tile([80, q * 2], BF16, name=f"accum{s}") for s in range(2)]
        for s in range(2):
            nc.vector.memset(acc[s][:], 0.0)

        # ---- add tiles (double buffered manually so the ones-channels are
        #      preset exactly once) ----
        addts = [add_pool.tile([80, chunk * 2], BF16, name=f"addtile{p}")
                 for p in range(2)]
        for p in range(2):
            nc.vector.memset(addts[p][:], 0.0)
            # ones channels 64..79: position t gets 1.0 at slot t%2
            # slot layout: value of row t lives at free offset 2t + (t%2)
            #   even t -> offset 4k, odd t -> offset 4k+3  (k = t//2)
            nc.vector.memset(addts[p][64:80, 0:chunk * 2:4], 1.0)
            nc.vector.memset(addts[p][64:80, 3:chunk * 2:4], 1.0)

        # ---- main loop ----
        for ci in range(n_chunk):
            addt = addts[ci % 2]
            for si in range(nsub):
                base = ci * chunk + si * SUB
                xt = xy_pool.tile([D_IN + 1, SUB], BF16)
                nc.sync.dma_start(out=xt[:], in_=xyt[:, base:base + SUB])
                for mi in range(nmm):
                    col0 = si * SUB + mi * MM  # within chunk
                    hp = ps1.tile([H0, MM], F32)
                    nc.tensor.matmul(hp[:], lhsT=w0t[:], rhs=xt[:, mi * MM:(mi + 1) * MM],
                                     start=True, stop=True)
                    hs = ht_pool.tile([H0, MM], BF16)
                    nc.scalar.activation(hs[:], hp[:], AF.Relu, bias=b0t[:, :])
                    zp = ps2.tile([H1, MM], F32)
                    nc.tensor.matmul(zp[:], lhsT=w1t[:], rhs=hs[:],
                                     start=True, stop=True)
                    # z1 -> addt with bias+relu, parity-interleaved slots
                    # even rows t: slot0 -> free offset 2t;  odd rows: slot1 -> 2t+1
                    # even cols of zp (t=col0+0,2,4..) -> addt offsets 2*(col0+2j)
                    o0 = 2 * col0
                    nc.vector.tensor_scalar(
                        out=addt[0:64, o0:o0 + 2 * MM:4],
                        in0=zp[:, 0:MM:2],
                        scalar1=b1t[:, :], scalar2=0.0,
                        op0=OP.add, op1=OP.max)
                    nc.vector.tensor_scalar(
                        out=addt[0:64, o0 + 3:o0 + 2 * MM:4],
                        in0=zp[:, 1:MM:2],
                        scalar1=b1t[:, :], scalar2=0.0,
                        op0=OP.add, op1=OP.max)
            for s in range(2):
                nc.gpsimd.scatter_add(
                    in_ap=acc[s][:, :],
                    idxs_ap=idst[s][:80, ci * (chunk // 16):(ci + 1) * (chunk // 16)],
                    add_ap=addt[:, :],
                    channels=80, num_elems=q, d=2, num_idxs=chunk)

        phase1.close()

        # ---- extraction + reduce-scatter ----
        head_pool = ctx.enter_context(tc.tile_pool(name="head", bufs=1))
        sx_pool = ctx.enter_context(tc.tile_pool(name="sx", bufs=3))
        psh = ctx.enter_context(tc.tile_pool(name="psh", bufs=1, space="PSUM"))
        rs_in = nc.dram_tensor("rs_in", [n_cores, 2, 65, qs], F32, kind="Internal")
        rs_out = nc.dram_tensor("rs_out", [2, 65, qs], F32, kind="Internal")
        for s in range(2):
            # DRAM layout: [shard, set, ch, qlocal]
            for g in range(n_cores):
                st = sx_pool.tile([65, qs], F32, tag="sext")
                nc.vector.tensor_tensor(
                    out=st[:],
                    in0=acc[s][0:65, 2 * g * qs:2 * (g + 1) * qs:2],
                    in1=acc[s][0:65, 2 * g * qs + 1:2 * (g + 1) * qs:2], op=OP.add)
                nc.sync.dma_start(out=rs_in[g, s], in_=st[:])
        nc.gpsimd.collective_compute(
            "ReduceScatter", OP.add,
            replica_groups=[list(range(n_cores))],
            ins=[rs_in[:, :, :, :]], outs=[rs_out[:, :, :]])

        # ---- head on owned q-shard ----
        stt = head_pool.tile([65, 2 * qs], F32, tag="stt")
        for s in range(2):
            nc.sync.dma_start(out=stt[:, s * qs:(s + 1) * qs], in_=rs_out[s])
        cl = head_pool.tile([1, 2 * qs], F32, tag="cl")
        nc.vector.tensor_scalar_max(cl[:], stt[64:65, :], 1.0)
        rec = head_pool.tile([1, 2 * qs], F32, tag="rec")
        nc.vector.reciprocal(rec[:], cl[:])
        recb = head_pool.tile([H1, 2 * qs], F32, tag="recb")
        for j in range(0, 2 * qs, MM):
            rp_ = psh.tile([H1, MM], F32, tag="recp")
            nc.tensor.matmul(rp_[:], lhsT=ones64[:], rhs=rec[:, j:j + MM],
                             start=True, stop=True)
            nc.vector.tensor_copy(out=recb[:, j:j + MM], in_=rp_[:])
        bt = head_pool.tile([H1, 2 * qs], F32, tag="bt")
        nc.vector.tensor_tensor(out=bt[:], in0=stt[0:64, :], in1=recb[:], op=OP.mult)

        projT = []  # meanT0, meanT1, lvT0, lvT1, sampT0, sampT1  ([D, qs])
        for s in range(2):
            mT = head_pool.tile([D, qs], F32, name=f"mT{s}")
            vT = head_pool.tile([D, qs], F32, name=f"vT{s}")
            for (wt, bt_, dst) in ((wmt[s], bmt[s], mT), (wvt[s], bvt[s], vT)):
                for j in range(0, qs, MM):
                    pp = psh.tile([D, MM], F32, tag="proj")
                    nc.tensor.matmul(pp[:], lhsT=wt[:], rhs=bt[:, s * qs + j:s * qs + j + MM],
                                     start=True, stop=True)
                    nc.vector.tensor_scalar(out=dst[:, j:j + MM], in0=pp[:],
                                            scalar1=bt_[:, :], scalar2=None,
                                            op0=OP.add)
            projT.append((mT, vT))
        sampT = []
        for s in range(2):
            mT, vT = projT[s]
            e = head_pool.tile([D, qs], F32, name=f"eT{s}")
            nc.scalar.activation(e[:], vT[:], AF.Exp, scale=0.5)
            sm = head_pool.tile([D, qs], F32, name=f"smT{s}")
            nc.vector.tensor_tensor(out=sm[:], in0=e[:], in1=epstt[s][:], op=OP.mult)
            nc.vector.tensor_tensor(out=sm[:], in0=sm[:], in1=mT[:], op=OP.add)
            sampT.append(sm)

        # ---- transpose back to natural layout + output ----
        slabs = [projT[0][0], projT[1][0], projT[0][1], projT[1][1],
                 sampT[0], sampT[1]]
        nt = qs // 128
        ost = head_pool.tile([128, 6 * nt * D], F32, tag="ost")
        for si_, src in enumerate(slabs):
            for t in range(nt):
                tp = psh.tile([128, D], F32, tag="otp")
                nc.tensor.transpose(tp[:], src[:, t * 128:(t + 1) * 128],
                                    ident[0:D, 0:D])
                o = (si_ * nt + t) * D
                nc.vector.tensor_copy(out=ost[:, o:o + D], in_=tp[:])
        for si_ in range(6):
            nc.sync.dma_start(
                out=out[si_].rearrange("(t p) d -> p t d", p=128),
                in_=ost[:, si_ * nt * D:(si_ + 1) * nt * D].rearrange(
                    "p (t d) -> p t d", d=D))

    nc.compile()
    return nc


_CACHE = {}


def _get_program():
    if "nc" not in _CACHE:
        _CACHE["nc"] = build_program()
    return _CACHE["nc"]


def _prep_inputs(X, y, z_ids0, z_ids1, W0, b0, W1, b1,
                 Wm0, bm0, Wv0, bv0, Wm1, bm1, Wv1, bv1, eps0, eps1,
                 n_cores=N_CORES, r=R, rp=RP, qs=QS):
    """Host-side data-independent prep: shard/pad/layout/dtype only."""
    bf16 = ml_dtypes.bfloat16
    xy = np.concatenate([np.asarray(X), np.asarray(y)], axis=1)  # [N, 65]
    xyt_full = np.ascontiguousarray(xy.T.astype(bf16))           # [65, N]

    in_maps = []
    for c in range(n_cores):
        lo, hi = c * r, (c + 1) * r
        m = {}
        xt = np.zeros((D_IN + 1, rp), dtype=bf16)
        xt[:, :r] = xyt_full[:, lo:hi]
        m["xyt"] = xt
        for s, ids in enumerate((z_ids0, z_ids1)):
            idp = np.full((rp,), -1, dtype=np.int16)
            idp[:r] = np.asarray(ids[lo:hi]).astype(np.int16)
            m[f"idsw{s}"] = np.ascontiguousarray(idp.reshape(rp // 16, 16).T)
        m["w0"] = np.asarray(W0).astype(bf16)
        m["b0"] = np.asarray(b0).astype(np.float32).reshape(H0, 1)
        m["w1"] = np.asarray(W1).astype(bf16)
        m["b1"] = np.asarray(b1).astype(np.float32).reshape(H1, 1)
        for s, (Wm, bm, Wv, bv, eps) in enumerate(
                ((Wm0, bm0, Wv0, bv0, eps0), (Wm1, bm1, Wv1, bv1, eps1))):
            m[f"wm{s}"] = np.asarray(Wm).astype(np.float32)
            m[f"bm{s}"] = np.asarray(bm).astype(np.float32).reshape(D, 1)
            m[f"wv{s}"] = np.asarray(Wv).astype(np.float32)
            m[f"bv{s}"] = np.asarray(bv).astype(np.float32).reshape(D, 1)
            m[f"epst{s}"] = np.ascontiguousarray(
                np.asarray(eps[c * qs:(c + 1) * qs]).astype(np.float32).T)
        in_maps.append(m)
    return in_maps


def kernel(**inputs):
    nc = _get_program()
    in_maps = _prep_inputs(**inputs)
    res = run_bass_kernel_spmd(nc, in_maps, core_ids=list(range(N_CORES)))
    shards = [res.results[c]["out"] for c in range(N_CORES)]
    return np.concatenate(shards, axis=1).astype(np.float32)


if __name__ == "__main__":
    nc = build_program()
    print("program built OK")
